# revision 38
# baseline (speedup 1.0000x reference)
"""Trainium2 Bass kernel for multi-head causal attention (GQA), 8-way tensor parallel.

Strategy (8 NeuronCores, one chip):
  - Shard heads: core c gets Q heads [c*HQ, (c+1)*HQ) and KV head group c.
  - All operands pre-cast to bf16 on the host and DMA'd via the hardware
    DGE queues (sync + scalar); gpsimd carries only collective triggers.
  - Transposed-domain attention: X^T feeds the projections so q^T/k^T land
    with head_dim on partitions, scores come out as S^T [k, q], softmax
    normalization is deferred (no max subtraction: |logits| < ~7), exp'd
    scores feed P@V directly, and the denominator rides a ones-vector
    matmul; 1/z via the fast custom-DVE reciprocal.
  - Software pipeline on the single PE queue: phase-A chunks 4-7 are pumped
    into head 0's ACT-paced attention; o_proj(h-1) is pumped into head h's
    attention; a tail budget of o_proj(2) matmuls is held back to cover the
    last AllToAll's latency.
  - o_proj: per-head AllToAll exchanges attention-output blocks so each core
    ends up with every core's head-h block for its own T/8 token slice;
    o_proj partials accumulate in SBUF across heads; output written bf16 and
    cast to f32 on the host, which also concatenates the 8 token slices.
"""

import sys

if "/opt/trn_rl_repo" not in sys.path:
    sys.path.insert(0, "/opt/trn_rl_repo")

import math
import numpy as np
import ml_dtypes

import concourse.bass as bass
import concourse.bass_isa as bass_isa
import concourse.bacc as bacc
import concourse.tile as tile
import concourse.mybir as mybir
from concourse.bass_utils import run_bass_kernel_spmd

P = 128
N_CORES = 8

FULL_CFG = dict(B=2, S=2048, E=4096, NH=32, NKV=8, HD=128)

CD = mybir.dt.bfloat16   # compute dtype for matmul operands
F32 = mybir.dt.float32
BF = ml_dtypes.bfloat16


def _derive(cfg):
    B, S, E, NH, NKV, HD = (cfg[k] for k in ("B", "S", "E", "NH", "NKV", "HD"))
    assert HD == P
    d = dict(cfg)
    d["T"] = B * S                    # total tokens (batch-major flatten)
    d["HQ"] = NH // N_CORES           # Q heads per core
    d["JK"] = d["HQ"] * HD            # joined_kv per core
    d["JKF"] = NH * HD                # full joined_kv
    d["EK"] = E // P                  # E k-tiles
    d["TCH"] = 512                    # phase-A token chunk
    d["SCH"] = 512                    # phase-B query chunk
    d["TSLICE"] = d["T"] // N_CORES   # tokens per core after AllToAll
    d["ST"] = S // P                  # key tiles per batch
    d["ECH"] = 512                    # o_proj output column chunk
    d["NQ"] = E // d["ECH"]           # o_proj column chunks
    assert d["HQ"] * NKV == NH or NH == NKV
    assert d["T"] % d["TCH"] == 0 and S % d["SCH"] == 0 and S % d["TCH"] == 0
    assert d["TSLICE"] % P == 0 and d["TSLICE"] == d["SCH"]
    return d


def build(cfg=None):
    """Build + compile the 8-core SPMD graph. Returns the Bacc module."""
    c = _derive(cfg or FULL_CFG)
    B, S, E, NH = c["B"], c["S"], c["E"], c["NH"]
    T, HQ, JK, JKF, EK = c["T"], c["HQ"], c["JK"], c["JKF"], c["EK"]
    TCH, SCH, TSLICE, ST = c["TCH"], c["SCH"], c["TSLICE"], c["ST"]
    ECH, NQ = c["ECH"], c["NQ"]
    NCH = T // TCH                   # phase-A chunks
    NSC = T // SCH                   # phase-B query chunks
    SCB = S // SCH                   # query chunks per batch
    MT = TSLICE // P                 # output row tiles per core
    inv_sqrt_hd = 1.0 / math.sqrt(c["HD"])
    NOFF = SCH // P                  # distinct diagonal mask offsets
    KG = max(EK // 4, 1)             # xt k-group per DMA

    nc = bacc.Bacc("TRN2", target_bir_lowering=False, debug=False,
                   num_devices=N_CORES)

    xt = nc.dram_tensor("xt", [P, NCH, EK, TCH], CD, kind="ExternalInput").ap()
    wq = nc.dram_tensor("wq", [P, EK, JK], CD, kind="ExternalInput").ap()
    wk = nc.dram_tensor("wk", [P, EK, P], CD, kind="ExternalInput").ap()
    wv = nc.dram_tensor("wv", [P, EK, P], CD, kind="ExternalInput").ap()
    wo = nc.dram_tensor("wo", [HQ, NQ, P, N_CORES * ECH], CD,
                        kind="ExternalInput").ap()
    masks = nc.dram_tensor("masks", [P, NOFF, SCH], CD, kind="ExternalInput").ap()
    ones_c = nc.dram_tensor("ones_c", [P, 1], CD, kind="ExternalInput").ap()
    ones_r = nc.dram_tensor("ones_r", [1, P], CD, kind="ExternalInput").ap()
    identity = nc.dram_tensor("identity", [P, P], CD, kind="ExternalInput").ap()
    out = nc.dram_tensor("out", [TSLICE, E], CD, kind="ExternalOutput").ap()

    with tile.TileContext(nc) as tc:
        with tc.tile_pool(name="const", bufs=1) as const, \
             tc.tile_pool(name="persist", bufs=1) as persist, \
             tc.tile_pool(name="dram", bufs=1, space="DRAM") as dram, \
             tc.tile_pool(name="eb", bufs=6) as eb, \
             tc.tile_pool(name="ob", bufs=4) as ob, \
             tc.tile_pool(name="zb", bufs=2) as zb, \
             tc.tile_pool(name="ps_s", bufs=3, space="PSUM") as ps_s, \
             tc.tile_pool(name="ps_o", bufs=2, space="PSUM") as ps_o:

            qT = persist.tile([P, HQ, T], CD)      # q^T: [d, head, token]
            kT = persist.tile([P, T], CD)          # k^T: [d, token]
            vN = persist.tile([P, T // P, P], CD)  # v natural: [t%128, t//128, d]

            a2a_in = []
            a2a_out = []
            for h in range(HQ):
                ain_h = dram.tile([N_CORES * P, TSLICE], CD, tag=f"ain{h}")
                aout_h = dram.tile([N_CORES * P, TSLICE], CD, tag=f"aout{h}")
                a2a_in.append(ain_h)
                a2a_out.append(aout_h)

            mask_sb = const.tile([P, NOFF, SCH], CD)
            ones_sb = const.tile([P, 1], CD)
            ones_row = const.tile([1, P], CD)
            ident = const.tile([P, P], CD)

            # ---- attention head emitter (phase B) ----
            def emit_attention_head(h, pump=None, pump_early=None,
                                    pump_from=0, pump_n=10, pump_budget=None,
                                    mid_cb=None, mid_at=None, chunks=None,
                                    fire_a2a=True):
                # pump: generator emitting fill-work (o_proj of the previous
                # head, or phase-A tail chunks) stepped between attention
                # tiles so the static PE order stays dense while ACT/DVE
                # chew on the softmax chain. pump_early: leftover fill-work
                # (o_proj of head h-2) for the sc < pump_from region, where
                # `pump` itself isn't safe yet (its AllToAll is in flight).
                state = {"left": pump_budget if pump_budget is not None
                         else (1 << 30)}

                def _pump(n):
                    if pump is None:
                        return
                    n = min(n, state["left"])
                    for _ in range(n):
                        if next(pump, "done") == "done":
                            state["left"] = 0
                            return
                        state["left"] -= 1

                def _pump_early(n):
                    if pump_early is None:
                        return
                    for _ in range(n):
                        if next(pump_early, "done") == "done":
                            return

                for sc in (chunks if chunks is not None else range(NSC)):
                    b = sc // SCB
                    jb = sc % SCB
                    s0 = sc * SCH            # global query token offset
                    s0b = jb * SCH           # within-batch offset
                    a = (s0b + SCH) // P     # active key tiles
                    o_ps = ps_o.tile([P, SCH], F32, tag="o")
                    zs_sb = zb.tile([P, SCH], CD, tag="zs")
                    for ti in range(a):
                        tg = b * ST + ti
                        off = ti * P - s0b
                        # diagonal tiles: queries below the tile's first key
                        # are fully masked — skip those columns. First/last
                        # tiles stay full width so the PSUM accumulation
                        # group starts and stops over the whole range.
                        lo = off if (0 < off and ti < a - 1) else 0
                        s_ps = ps_s.tile([P, SCH], F32, tag="s")
                        nc.tensor.matmul(s_ps[:, lo:],
                                         kT[:, tg * P:(tg + 1) * P],
                                         qT[:, h, s0 + lo:s0 + SCH],
                                         start=True, stop=True)
                        e_sb = eb.tile([P, SCH], CD, tag="e")
                        nc.scalar.activation(e_sb[:, lo:], s_ps[:, lo:],
                                             mybir.ActivationFunctionType.Exp,
                                             scale=inv_sqrt_hd)
                        if off >= 0:  # diagonal tile: zero the future keys
                            nc.vector.tensor_mul(e_sb[:, lo:], e_sb[:, lo:],
                                                 mask_sb[:, off // P, lo:])
                        nc.tensor.matmul(o_ps[:, lo:], vN[:, tg, :],
                                         e_sb[:, lo:],
                                         start=(ti == 0), stop=(ti == a - 1))
                        # bf16 running sum of exp'd scores (keys on partitions)
                        if ti == 0:
                            nc.vector.tensor_copy(zs_sb[:], e_sb[:])
                        else:
                            nc.vector.tensor_add(zs_sb[:, lo:], zs_sb[:, lo:],
                                                 e_sb[:, lo:])
                        if sc >= pump_from:
                            _pump(pump_n)
                        else:
                            _pump_early(pump_n)
                    # per-query denominator: all-reduce the exp-sums across
                    # partitions on the (idle) gpsimd engine — replaces a PE
                    # ones-matmul + cast + broadcast — then 1/z on fast DVE
                    zbc_f = zb.tile([P, SCH], F32, tag="zr")
                    nc.gpsimd.partition_all_reduce(
                        zbc_f[:], zs_sb[:], channels=P,
                        reduce_op=bass_isa.ReduceOp.add)
                    zi_f = zb.tile([P, SCH], F32, tag="zi")
                    nc.vector.reciprocal_approx_fast(zi_f[:], zbc_f[:])
                    o_sb = ob.tile([P, SCH], CD, tag="osb")
                    nc.vector.tensor_mul(o_sb[:], o_ps[:], zi_f[:])
                    # SCH == TSLICE: chunk sc is exactly a2a block sc
                    nc.sync.dma_start(a2a_in[h][sc * P:(sc + 1) * P, :], o_sb[:])
                    if mid_cb is not None and sc == mid_at:
                        mid_cb(o_sb)

                if fire_a2a:
                    nc.gpsimd.collective_compute(
                        "AllToAll", mybir.AluOpType.bypass,
                        ins=[a2a_in[h].opt()], outs=[a2a_out[h].opt()],
                        replica_groups=[list(range(N_CORES))])

            # ---- Region 1: projections (phase A) + head-0 attention ----
            with tc.tile_pool(name="wpool", bufs=1) as wpool, \
                 tc.tile_pool(name="xpool", bufs=2) as xpool, \
                 tc.tile_pool(name="vstage", bufs=2) as vstage, \
                 tc.tile_pool(name="pa", bufs=2, space="PSUM") as pa, \
                 tc.tile_pool(name="pt", bufs=1, space="PSUM") as pt:

                wq_sb = wpool.tile([P, EK, JK], CD, tag="w")
                wk_sb = wpool.tile([P, EK, P], CD, tag="wk")
                wv_sb = wpool.tile([P, EK, P], CD, tag="wv")
                # few, large enqueues (each dma_start instruction costs the
                # issuing engine ~0.7us), ordered to match consumption:
                # K pass first, then V, then Q heads
                EQ = EK // 4
                nc.scalar.dma_start(wk_sb[:, :EQ, :], wk[:, :EQ, :])
                nc.scalar.dma_start(wv_sb[:, :EQ, :], wv[:, :EQ, :])
                nc.scalar.dma_start(wq_sb[:, :EQ // 2, :], wq[:, :EQ // 2, :])
                nc.scalar.dma_start(wk_sb[:, EQ:2 * EQ, :], wk[:, EQ:2 * EQ, :])
                nc.scalar.dma_start(wv_sb[:, EQ:2 * EQ, :], wv[:, EQ:2 * EQ, :])
                nc.scalar.dma_start(wk_sb[:, 2 * EQ:, :], wk[:, 2 * EQ:, :])
                nc.scalar.dma_start(wv_sb[:, 2 * EQ:, :], wv[:, 2 * EQ:, :])
                nc.scalar.dma_start(wq_sb[:, EQ // 2:2 * EQ, :],
                                    wq[:, EQ // 2:2 * EQ, :])
                nc.scalar.dma_start(mask_sb[:], masks)
                nc.scalar.dma_start(ones_sb[:], ones_c)
                nc.scalar.dma_start(ones_row[:], ones_r)
                nc.scalar.dma_start(ident[:], identity)
                nc.scalar.dma_start(wq_sb[:, 2 * EQ:, :], wq[:, 2 * EQ:, :])

                # tiny warmup AllToAll: absorbs the one-time ncfw setup +
                # cross-core rendezvous barrier while phase A computes
                warm_in = dram.tile([N_CORES, 64], CD, tag="warm_in")
                warm_out = dram.tile([N_CORES, 64], CD, tag="warm_out")
                nc.gpsimd.collective_compute(
                    "AllToAll", mybir.AluOpType.bypass,
                    ins=[warm_in.opt()], outs=[warm_out.opt()],
                    replica_groups=[list(range(N_CORES))])

                xt_tiles = {}

                def load_chunk(ch):
                    xt_t = xpool.tile([P, EK, TCH], CD, tag="x")
                    if ch == 0:  # split so the first k-tiles land ASAP
                        for k0, k1 in ((0, 4), (4, 8), (8, 20), (20, EK)):
                            nc.sync.dma_start(xt_t[:, k0:k1, :],
                                              xt[:, ch, k0:k1, :])
                    else:
                        nc.sync.dma_start(xt_t[:], xt[:, ch])
                    xt_tiles[ch] = xt_t

                def a_chunk_gen(ch):
                    # yields after each PE instruction; issues the NEXT
                    # chunk's DMA up front for lead time. K and V run FIRST
                    # so attention on this chunk's keys unblocks after only
                    # ~half the chunk (qT of head 0 right after).
                    if ch + 1 < NCH and ch + 1 not in xt_tiles:
                        load_chunk(ch + 1)
                    xt_t = xt_tiles.pop(ch)
                    t0 = ch * TCH
                    # NOTE: each pass's finalizing copy is emitted BEFORE its
                    # last yield — code after a yield only runs on the next
                    # resume, so a pump budget boundary would otherwise leave
                    # the SBUF result unwritten when its readers are emitted
                    acc = pa.tile([P, TCH], F32, tag="acc")
                    for k in range(EK):
                        nc.tensor.matmul(acc[:], wk_sb[:, k, :], xt_t[:, k, :],
                                         start=(k == 0), stop=(k == EK - 1))
                        if k < EK - 1:
                            yield
                    nc.vector.tensor_copy(kT[:, t0:t0 + TCH], acc[:])
                    yield
                    acc = pa.tile([P, TCH], F32, tag="acc")
                    for k in range(EK):
                        nc.tensor.matmul(acc[:], wv_sb[:, k, :], xt_t[:, k, :],
                                         start=(k == 0), stop=(k == EK - 1))
                        if k < EK - 1:
                            yield
                    vt_sb = vstage.tile([P, TCH], CD, tag="vt")
                    nc.vector.tensor_copy(vt_sb[:], acc[:])
                    yield
                    for i in range(TCH // P):
                        ps = pt.tile([P, P], CD, tag="tr")
                        nc.tensor.transpose(ps[:], vt_sb[:, i * P:(i + 1) * P],
                                            ident[:])
                        nc.vector.tensor_copy(vN[:, (t0 // P) + i, :], ps[:])
                        yield
                    for h in range(HQ):
                        acc = pa.tile([P, TCH], F32, tag="acc")
                        for k in range(EK):
                            nc.tensor.matmul(acc[:],
                                             wq_sb[:, k, h * P:(h + 1) * P],
                                             xt_t[:, k, :],
                                             start=(k == 0), stop=(k == EK - 1))
                            if k < EK - 1:
                                yield
                        nc.vector.tensor_copy(qT[:, h, t0:t0 + TCH], acc[:])
                        yield

                def drain(gen):
                    if gen is not None:
                        for _ in gen:
                            pass

                load_chunk(0)
                for ch in range(2):
                    drain(a_chunk_gen(ch))

                # second warmup: fires just before the first real AllToAll so
                # the collective engine doesn't re-idle in between
                warm2_in = dram.tile([N_CORES, 64], CD, tag="warm2_in")
                warm2_out = dram.tile([N_CORES, 64], CD, tag="warm2_out")
                nc.gpsimd.collective_compute(
                    "AllToAll", mybir.AluOpType.bypass,
                    ins=[warm2_in.opt()], outs=[warm2_out.opt()],
                    replica_groups=[list(range(N_CORES))])

                def a_tail():
                    for ch in range(2, NCH):
                        yield from a_chunk_gen(ch)

                def warm3(o_sb):
                    # data-dependent warmup: copying a sliver of B(0)'s
                    # mid-head output into the collective input delays the
                    # trigger until B(0) is actually half done on hardware,
                    # keeping the CC engines warm right before AllToAll(0)
                    w3i = dram.tile([N_CORES, 64], CD, tag="warm3_in")
                    w3o = dram.tile([N_CORES, 64], CD, tag="warm3_out")
                    nc.sync.dma_start(w3i[:], o_sb[0:N_CORES, 0:64])
                    nc.gpsimd.collective_compute(
                        "AllToAll", mybir.AluOpType.bypass,
                        ins=[w3i.opt()], outs=[w3o.opt()],
                        replica_groups=[list(range(N_CORES))])

                # pump_n=18 keeps every A-chunk's emission strictly ahead of
                # the B(0) chunks that read it (K/V/Q0 land in the first 100
                # yields of each 196-yield chunk); budget 1080 holds back the
                # final Q1-3 passes of chunk 7 (96 yields, not needed by B(0))
                # as early fill for B(1)'s first half
                tail = a_tail()
                emit_attention_head(0, pump=tail, pump_from=0, pump_n=18,
                                    pump_budget=1080, mid_cb=warm3, mid_at=5)
                # B(1) chunks 0-3: only needs qT[h=1]/kT/vN of chunks 0-3;
                # runs here so the A-tail leftover can fill its PE idle
                # (o_proj(0) can't — its AllToAll is still in flight)
                emit_attention_head(1, pump_early=tail, pump_from=4,
                                    chunks=range(0, SCB), fire_a2a=False)
                drain(tail)

            # ---- Region 2: heads 1-3 + o_proj pipeline (phase C) ----
            with tc.tile_pool(name="otp", bufs=2) as otp, \
                 tc.tile_pool(name="wop", bufs=9) as wop, \
                 tc.tile_pool(name="oaccp", bufs=1) as oaccp, \
                 tc.tile_pool(name="pc", bufs=3, space="PSUM") as pc:

                out_acc = oaccp.tile([P, MT, E], CD)

                def emit_oproj_loads(h):
                    wons = []
                    for q in range(NQ):
                        won_q = wop.tile([P, N_CORES, ECH], CD, tag="wo")
                        nc.sync.dma_start(won_q[:], wo[h, q])
                        wons.append(won_q)
                    return wons

                def emit_ot(h):
                    # gather this head's AllToAll result, split per m-tile so
                    # the first o_proj matmul only waits for a quarter of the
                    # data. The sync engine stalls on the collective's
                    # semaphore at enqueue time — ordering below ensures
                    # nothing urgent sits behind it in the sync queue.
                    ot_h = otp.tile([P, N_CORES, TSLICE], CD, tag="ot")
                    a2a_r = a2a_out[h][:].rearrange("(j p) t -> p j t", p=P)
                    for m in range(MT):
                        nc.sync.dma_start(ot_h[:, :, m * P:(m + 1) * P],
                                          a2a_r[:, :, m * P:(m + 1) * P])
                    return ot_h

                def emit_oproj_head(h, ot_h, wons, m_outer=False):
                    # q-chunks processed in pairs: within a (jj, m) step the
                    # two matmuls share the same stationary (the ot slice), so
                    # the compiler can skip the second LDWEIGHTS.
                    # m-outer (last head): each output row-tile finishes early
                    # so its final out-DMA overlaps the remaining matmuls
                    if m_outer:
                        order = [(q2, m) for m in range(MT)
                                 for q2 in range(0, NQ, 2)]
                    else:
                        order = [(q2, m) for q2 in range(0, NQ, 2)
                                 for m in range(MT)]

                    def gen():
                        for q2, m in order:
                            acc0 = pc.tile([P, ECH], F32, tag="c")
                            acc1 = pc.tile([P, ECH], F32, tag="c")
                            for jj in range(N_CORES):
                                ot_s = ot_h[:, jj, m * P:(m + 1) * P]
                                nc.tensor.matmul(
                                    acc0[:], ot_s, wons[q2][:, jj, :],
                                    start=(jj == 0), stop=(jj == N_CORES - 1))
                                yield
                                nc.tensor.matmul(
                                    acc1[:], ot_s, wons[q2 + 1][:, jj, :],
                                    start=(jj == 0), stop=(jj == N_CORES - 1))
                                yield
                            for i, acc_c in enumerate((acc0, acc1)):
                                q = q2 + i
                                if h == 0:
                                    nc.vector.tensor_copy(
                                        out_acc[:, m, q * ECH:(q + 1) * ECH],
                                        acc_c[:])
                                else:
                                    nc.vector.tensor_add(
                                        out_acc[:, m, q * ECH:(q + 1) * ECH],
                                        out_acc[:, m, q * ECH:(q + 1) * ECH],
                                        acc_c[:])
                                yield
                            # out-row halves fire as soon as their q-chunks
                            # are done, shrinking the post-compute DMA tail
                            if m_outer and q2 == NQ // 2 - 2:
                                nc.sync.dma_start(
                                    out[m * P:(m + 1) * P, :E // 2],
                                    out_acc[:, m, :E // 2])
                            if m_outer and q2 == NQ - 2:
                                nc.sync.dma_start(
                                    out[m * P:(m + 1) * P, E // 2:],
                                    out_acc[:, m, E // 2:])

                    return gen()

                def drain2(gen):
                    for _ in gen:
                        pass

                # Each C(h-1) gen is pumped into B(h) sc>=4 up to a budget;
                # its leftover becomes the early fill for B(h+1) sc<4 (where
                # C(h)'s own AllToAll is still in flight), and C(2)'s bigger
                # leftover covers the final AllToAll's latency.
                ot_p = emit_ot(0)       # AllToAll(0) already done: no stall
                wons = emit_oproj_loads(0)
                gen = emit_oproj_head(0, ot_p, wons)
                wons = emit_oproj_loads(1)
                # B(1) second half (first half ran under the A-tail leftover)
                emit_attention_head(1, pump=gen, pump_from=SCB + 1, pump_n=10,
                                    pump_budget=220, chunks=range(SCB, NSC))
                early = gen
                for h in range(2, HQ):
                    ot_p = emit_ot(h - 1)
                    gen = emit_oproj_head(h - 1, ot_p, wons)
                    wons = emit_oproj_loads(h)
                    budget = 130 if h == HQ - 1 else 220
                    emit_attention_head(h, pump=gen, pump_early=early,
                                        pump_from=5, pump_n=10,
                                        pump_budget=budget)
                    early = gen
                drain2(early)  # C(2) leftover runs while AllToAll(3) flies
                ot_p = emit_ot(HQ - 1)
                drain2(emit_oproj_head(HQ - 1, ot_p, wons, m_outer=True))

    nc.compile()
    return nc, c


def _make_masks(cfg):
    c = _derive(cfg)
    SCH = c["SCH"]
    NOFF = SCH // P
    m = np.zeros((NOFF, P, SCH), np.float32)
    for o in range(NOFF):
        for p in range(P):
            lo = p + o * P
            if lo < SCH:
                m[o, p, lo:] = 1.0
    return np.ascontiguousarray(m.transpose(1, 0, 2)).astype(BF)


def make_in_maps(cfg, hidden_states, Wq, Wk, Wv, Wo):
    c = _derive(cfg)
    B, S, E, NH, HQ = c["B"], c["S"], c["E"], c["NH"], c["HQ"]
    T, EK, TCH, ECH, NQ = c["T"], c["EK"], c["TCH"], c["ECH"], c["NQ"]
    NCH = T // TCH
    # X^T tiled [p, chunk, k, t'] so each chunk load is per-partition linear
    h2 = np.asarray(hidden_states, np.float32).reshape(T, E).astype(BF)
    xt_c = np.ascontiguousarray(
        h2.reshape(NCH, TCH, EK, P).transpose(3, 0, 2, 1))
    Wq = np.asarray(Wq, np.float32).astype(BF)
    Wk = np.asarray(Wk, np.float32).astype(BF)
    Wv = np.asarray(Wv, np.float32).astype(BF)
    # Wo [j*HQ*P + h*P + p, q*ECH + e'] -> [h, q, p, j*ECH + e']
    wo_c = np.ascontiguousarray(
        np.asarray(Wo, np.float32).astype(BF)
        .reshape(N_CORES, HQ, P, NQ, ECH).transpose(1, 3, 2, 0, 4)
        .reshape(HQ, NQ, P, N_CORES * ECH))
    masks = _make_masks(cfg)
    ones_col = np.ones((P, 1), BF)
    ones_row = np.ones((1, P), BF)
    ident = np.eye(P, dtype=np.float32).astype(BF)
    nkv_per = max(c["NKV"] // N_CORES, 1)
    in_maps = []
    for cidx in range(N_CORES):
        wq_c = np.ascontiguousarray(
            Wq[:, cidx * HQ:(cidx + 1) * HQ, :]
            .reshape(EK, P, HQ * P).transpose(1, 0, 2))
        wk_c = np.ascontiguousarray(
            Wk[:, cidx * nkv_per, :].reshape(EK, P, P).transpose(1, 0, 2))
        wv_c = np.ascontiguousarray(
            Wv[:, cidx * nkv_per, :].reshape(EK, P, P).transpose(1, 0, 2))
        in_maps.append({
            "xt": xt_c,
            "wq": wq_c,
            "wk": wk_c,
            "wv": wv_c,
            "wo": wo_c,
            "masks": masks,
            "ones_c": ones_col,
            "ones_r": ones_row,
            "identity": ident,
        })
    return in_maps


_CACHE = {}


def _get_built(key, cfg):
    if key not in _CACHE:
        _CACHE[key] = build(cfg)
    return _CACHE[key]


def kernel(hidden_states, Wq, Wk, Wv, Wo):
    cfg = FULL_CFG
    nc, c = _get_built("full", cfg)
    in_maps = make_in_maps(cfg, hidden_states, Wq, Wk, Wv, Wo)
    res = run_bass_kernel_spmd(nc, in_maps, core_ids=list(range(N_CORES)))
    outs = [np.asarray(res.results[i]["out"]).astype(np.float32)
            for i in range(N_CORES)]
    full = np.concatenate(outs, axis=0)
    return full.reshape(c["B"], c["S"], c["E"])


# revision 39
# speedup vs baseline: 1.0172x; 1.0172x over previous
"""Trainium2 Bass kernel for multi-head causal attention (GQA), 8-way tensor parallel.

Strategy (8 NeuronCores, one chip):
  - Shard heads: core c gets Q heads [c*HQ, (c+1)*HQ) and KV head group c.
  - All operands pre-cast to bf16 on the host and DMA'd via the hardware
    DGE queues (sync + scalar); gpsimd carries only collective triggers.
  - Transposed-domain attention: X^T feeds the projections so q^T/k^T land
    with head_dim on partitions, scores come out as S^T [k, q], softmax
    normalization is deferred (no max subtraction: |logits| < ~7), exp'd
    scores feed P@V directly, and the denominator rides a ones-vector
    matmul; 1/z via the fast custom-DVE reciprocal.
  - Software pipeline on the single PE queue: phase-A chunks 4-7 are pumped
    into head 0's ACT-paced attention; o_proj(h-1) is pumped into head h's
    attention; a tail budget of o_proj(2) matmuls is held back to cover the
    last AllToAll's latency.
  - o_proj: per-head AllToAll exchanges attention-output blocks so each core
    ends up with every core's head-h block for its own T/8 token slice;
    o_proj partials accumulate in SBUF across heads; output written bf16 and
    cast to f32 on the host, which also concatenates the 8 token slices.
"""

import sys

if "/opt/trn_rl_repo" not in sys.path:
    sys.path.insert(0, "/opt/trn_rl_repo")

import math
import numpy as np
import ml_dtypes

import concourse.bass as bass
import concourse.bacc as bacc
import concourse.tile as tile
import concourse.mybir as mybir
from concourse.bass_utils import run_bass_kernel_spmd

P = 128
N_CORES = 8

FULL_CFG = dict(B=2, S=2048, E=4096, NH=32, NKV=8, HD=128)

CD = mybir.dt.bfloat16   # compute dtype for matmul operands
F32 = mybir.dt.float32
BF = ml_dtypes.bfloat16


def _derive(cfg):
    B, S, E, NH, NKV, HD = (cfg[k] for k in ("B", "S", "E", "NH", "NKV", "HD"))
    assert HD == P
    d = dict(cfg)
    d["T"] = B * S                    # total tokens (batch-major flatten)
    d["HQ"] = NH // N_CORES           # Q heads per core
    d["JK"] = d["HQ"] * HD            # joined_kv per core
    d["JKF"] = NH * HD                # full joined_kv
    d["EK"] = E // P                  # E k-tiles
    d["TCH"] = 512                    # phase-A token chunk
    d["SCH"] = 512                    # phase-B query chunk
    d["TSLICE"] = d["T"] // N_CORES   # tokens per core after AllToAll
    d["ST"] = S // P                  # key tiles per batch
    d["ECH"] = 512                    # o_proj output column chunk
    d["NQ"] = E // d["ECH"]           # o_proj column chunks
    assert d["HQ"] * NKV == NH or NH == NKV
    assert d["T"] % d["TCH"] == 0 and S % d["SCH"] == 0 and S % d["TCH"] == 0
    assert d["TSLICE"] % P == 0 and d["TSLICE"] == d["SCH"]
    return d


def build(cfg=None):
    """Build + compile the 8-core SPMD graph. Returns the Bacc module."""
    c = _derive(cfg or FULL_CFG)
    B, S, E, NH = c["B"], c["S"], c["E"], c["NH"]
    T, HQ, JK, JKF, EK = c["T"], c["HQ"], c["JK"], c["JKF"], c["EK"]
    TCH, SCH, TSLICE, ST = c["TCH"], c["SCH"], c["TSLICE"], c["ST"]
    ECH, NQ = c["ECH"], c["NQ"]
    NCH = T // TCH                   # phase-A chunks
    NSC = T // SCH                   # phase-B query chunks
    SCB = S // SCH                   # query chunks per batch
    MT = TSLICE // P                 # output row tiles per core
    inv_sqrt_hd = 1.0 / math.sqrt(c["HD"])
    NOFF = SCH // P                  # distinct diagonal mask offsets
    KG = max(EK // 4, 1)             # xt k-group per DMA

    nc = bacc.Bacc("TRN2", target_bir_lowering=False, debug=False,
                   num_devices=N_CORES)

    xt = nc.dram_tensor("xt", [P, NCH, EK, TCH], CD, kind="ExternalInput").ap()
    wq = nc.dram_tensor("wq", [P, EK, JK], CD, kind="ExternalInput").ap()
    wk = nc.dram_tensor("wk", [P, EK, P], CD, kind="ExternalInput").ap()
    wv = nc.dram_tensor("wv", [P, EK, P], CD, kind="ExternalInput").ap()
    wo = nc.dram_tensor("wo", [HQ, NQ, P, N_CORES * ECH], CD,
                        kind="ExternalInput").ap()
    masks = nc.dram_tensor("masks", [P, NOFF, SCH], CD, kind="ExternalInput").ap()
    ones_c = nc.dram_tensor("ones_c", [P, 1], CD, kind="ExternalInput").ap()
    ones_r = nc.dram_tensor("ones_r", [1, P], CD, kind="ExternalInput").ap()
    identity = nc.dram_tensor("identity", [P, P], CD, kind="ExternalInput").ap()
    out = nc.dram_tensor("out", [TSLICE, E], CD, kind="ExternalOutput").ap()

    with tile.TileContext(nc) as tc:
        with tc.tile_pool(name="const", bufs=1) as const, \
             tc.tile_pool(name="persist", bufs=1) as persist, \
             tc.tile_pool(name="dram", bufs=1, space="DRAM") as dram, \
             tc.tile_pool(name="eb", bufs=6) as eb, \
             tc.tile_pool(name="ob", bufs=4) as ob, \
             tc.tile_pool(name="zb", bufs=2) as zb, \
             tc.tile_pool(name="ps_s", bufs=2, space="PSUM") as ps_s, \
             tc.tile_pool(name="ps_o", bufs=2, space="PSUM") as ps_o, \
             tc.tile_pool(name="ps_zz", bufs=1, space="PSUM") as ps_zz:

            qT = persist.tile([P, HQ, T], CD)      # q^T: [d, head, token]
            kT = persist.tile([P, T], CD)          # k^T: [d, token]
            vN = persist.tile([P, T // P, P], CD)  # v natural: [t%128, t//128, d]

            a2a_in = []
            a2a_out = []
            for h in range(HQ):
                ain_h = dram.tile([N_CORES * P, TSLICE], CD, tag=f"ain{h}")
                aout_h = dram.tile([N_CORES * P, TSLICE], CD, tag=f"aout{h}")
                a2a_in.append(ain_h)
                a2a_out.append(aout_h)

            mask_sb = const.tile([P, NOFF, SCH], CD)
            ones_sb = const.tile([P, 1], CD)
            ones_row = const.tile([1, P], CD)
            ident = const.tile([P, P], CD)

            # ---- attention head emitter (phase B) ----
            def emit_attention_head(h, pump=None, pump_early=None,
                                    pump_from=0, pump_n=10, pump_budget=None,
                                    mid_cb=None, mid_at=None, chunks=None,
                                    fire_a2a=True):
                # pump: generator emitting fill-work (o_proj of the previous
                # head, or phase-A tail chunks) stepped between attention
                # tiles so the static PE order stays dense while ACT/DVE
                # chew on the softmax chain. pump_early: leftover fill-work
                # (o_proj of head h-2) for the sc < pump_from region, where
                # `pump` itself isn't safe yet (its AllToAll is in flight).
                state = {"left": pump_budget if pump_budget is not None
                         else (1 << 30)}

                def _pump(n):
                    if pump is None:
                        return
                    n = min(n, state["left"])
                    for _ in range(n):
                        if next(pump, "done") == "done":
                            state["left"] = 0
                            return
                        state["left"] -= 1

                def _pump_early(n):
                    if pump_early is None:
                        return
                    for _ in range(n):
                        if next(pump_early, "done") == "done":
                            return

                for sc in (chunks if chunks is not None else range(NSC)):
                    b = sc // SCB
                    jb = sc % SCB
                    s0 = sc * SCH            # global query token offset
                    s0b = jb * SCH           # within-batch offset
                    a = (s0b + SCH) // P     # active key tiles
                    o_ps = ps_o.tile([P, SCH], F32, tag="o")
                    zs_sb = zb.tile([P, SCH], CD, tag="zs")
                    for ti in range(a):
                        tg = b * ST + ti
                        off = ti * P - s0b
                        # diagonal tiles: queries below the tile's first key
                        # are fully masked — skip those columns. First/last
                        # tiles stay full width so the PSUM accumulation
                        # group starts and stops over the whole range.
                        lo = off if (0 < off and ti < a - 1) else 0
                        s_ps = ps_s.tile([P, SCH], F32, tag="s")
                        nc.tensor.matmul(s_ps[:, lo:],
                                         kT[:, tg * P:(tg + 1) * P],
                                         qT[:, h, s0 + lo:s0 + SCH],
                                         start=True, stop=True)
                        e_sb = eb.tile([P, SCH], CD, tag="e")
                        nc.scalar.activation(e_sb[:, lo:], s_ps[:, lo:],
                                             mybir.ActivationFunctionType.Exp,
                                             scale=inv_sqrt_hd)
                        if off >= 0:  # diagonal tile: zero the future keys
                            nc.vector.tensor_mul(e_sb[:, lo:], e_sb[:, lo:],
                                                 mask_sb[:, off // P, lo:])
                        nc.tensor.matmul(o_ps[:, lo:], vN[:, tg, :],
                                         e_sb[:, lo:],
                                         start=(ti == 0), stop=(ti == a - 1))
                        # bf16 running sum of exp'd scores (keys on partitions)
                        if ti == 0:
                            nc.vector.tensor_copy(zs_sb[:], e_sb[:])
                        else:
                            nc.vector.tensor_add(zs_sb[:, lo:], zs_sb[:, lo:],
                                                 e_sb[:, lo:])
                        if sc >= pump_from:
                            _pump(pump_n)
                        else:
                            _pump_early(pump_n)
                    # z = 1^T @ zs  (per-query denominator), 1/z on fast DVE
                    z_ps = ps_zz.tile([1, SCH], F32, tag="zz")
                    nc.tensor.matmul(z_ps[:], ones_sb[:], zs_sb[:],
                                     start=True, stop=True)
                    zi_sb = zb.tile([1, SCH], F32, tag="zi")
                    nc.vector.reciprocal_approx_fast(zi_sb[:], z_ps[:])
                    zi_cd = zb.tile([1, SCH], CD, tag="zic")
                    nc.vector.tensor_copy(zi_cd[:], zi_sb[:])
                    # broadcast 1/z across partitions on the (idle) gpsimd
                    # engine instead of a PE ones-matmul + ACT copy
                    zbc_sb = zb.tile([P, SCH], CD, tag="zbc_sb")
                    nc.gpsimd.partition_broadcast(zbc_sb[:], zi_cd[:])
                    o_sb = ob.tile([P, SCH], CD, tag="osb")
                    nc.vector.tensor_mul(o_sb[:], o_ps[:], zbc_sb[:])
                    # SCH == TSLICE: chunk sc is exactly a2a block sc
                    nc.sync.dma_start(a2a_in[h][sc * P:(sc + 1) * P, :], o_sb[:])
                    if mid_cb is not None and sc == mid_at:
                        mid_cb(o_sb)

                if fire_a2a:
                    nc.gpsimd.collective_compute(
                        "AllToAll", mybir.AluOpType.bypass,
                        ins=[a2a_in[h].opt()], outs=[a2a_out[h].opt()],
                        replica_groups=[list(range(N_CORES))])

            # ---- Region 1: projections (phase A) + head-0 attention ----
            with tc.tile_pool(name="wpool", bufs=1) as wpool, \
                 tc.tile_pool(name="xpool", bufs=2) as xpool, \
                 tc.tile_pool(name="vstage", bufs=2) as vstage, \
                 tc.tile_pool(name="pa", bufs=2, space="PSUM") as pa, \
                 tc.tile_pool(name="pt", bufs=1, space="PSUM") as pt:

                wq_sb = wpool.tile([P, EK, JK], CD, tag="w")
                wk_sb = wpool.tile([P, EK, P], CD, tag="wk")
                wv_sb = wpool.tile([P, EK, P], CD, tag="wv")
                # few, large enqueues (each dma_start instruction costs the
                # issuing engine ~0.7us), ordered to match consumption:
                # K pass first, then V, then Q heads
                EQ = EK // 4
                nc.scalar.dma_start(wk_sb[:, :EQ, :], wk[:, :EQ, :])
                nc.scalar.dma_start(wv_sb[:, :EQ, :], wv[:, :EQ, :])
                nc.scalar.dma_start(wq_sb[:, :EQ // 2, :], wq[:, :EQ // 2, :])
                nc.scalar.dma_start(wk_sb[:, EQ:2 * EQ, :], wk[:, EQ:2 * EQ, :])
                nc.scalar.dma_start(wv_sb[:, EQ:2 * EQ, :], wv[:, EQ:2 * EQ, :])
                nc.scalar.dma_start(wk_sb[:, 2 * EQ:, :], wk[:, 2 * EQ:, :])
                nc.scalar.dma_start(wv_sb[:, 2 * EQ:, :], wv[:, 2 * EQ:, :])
                nc.scalar.dma_start(wq_sb[:, EQ // 2:2 * EQ, :],
                                    wq[:, EQ // 2:2 * EQ, :])
                nc.scalar.dma_start(mask_sb[:], masks)
                nc.scalar.dma_start(ones_sb[:], ones_c)
                nc.scalar.dma_start(ones_row[:], ones_r)
                nc.scalar.dma_start(ident[:], identity)
                nc.scalar.dma_start(wq_sb[:, 2 * EQ:, :], wq[:, 2 * EQ:, :])

                # tiny warmup AllToAll: absorbs the one-time ncfw setup +
                # cross-core rendezvous barrier while phase A computes
                warm_in = dram.tile([N_CORES, 64], CD, tag="warm_in")
                warm_out = dram.tile([N_CORES, 64], CD, tag="warm_out")
                nc.gpsimd.collective_compute(
                    "AllToAll", mybir.AluOpType.bypass,
                    ins=[warm_in.opt()], outs=[warm_out.opt()],
                    replica_groups=[list(range(N_CORES))])

                xt_tiles = {}

                def load_chunk(ch):
                    xt_t = xpool.tile([P, EK, TCH], CD, tag="x")
                    if ch == 0:  # split so the first k-tiles land ASAP
                        for k0, k1 in ((0, 4), (4, 8), (8, 20), (20, EK)):
                            nc.sync.dma_start(xt_t[:, k0:k1, :],
                                              xt[:, ch, k0:k1, :])
                    else:
                        nc.sync.dma_start(xt_t[:], xt[:, ch])
                    xt_tiles[ch] = xt_t

                def a_chunk_gen(ch):
                    # yields after each PE instruction; issues the NEXT
                    # chunk's DMA up front for lead time. K and V run FIRST
                    # so attention on this chunk's keys unblocks after only
                    # ~half the chunk (qT of head 0 right after).
                    if ch + 1 < NCH and ch + 1 not in xt_tiles:
                        load_chunk(ch + 1)
                    xt_t = xt_tiles.pop(ch)
                    t0 = ch * TCH
                    # NOTE: each pass's finalizing copy is emitted BEFORE its
                    # last yield — code after a yield only runs on the next
                    # resume, so a pump budget boundary would otherwise leave
                    # the SBUF result unwritten when its readers are emitted
                    acc = pa.tile([P, TCH], F32, tag="acc")
                    for k in range(EK):
                        nc.tensor.matmul(acc[:], wk_sb[:, k, :], xt_t[:, k, :],
                                         start=(k == 0), stop=(k == EK - 1))
                        if k < EK - 1:
                            yield
                    nc.vector.tensor_copy(kT[:, t0:t0 + TCH], acc[:])
                    yield
                    acc = pa.tile([P, TCH], F32, tag="acc")
                    for k in range(EK):
                        nc.tensor.matmul(acc[:], wv_sb[:, k, :], xt_t[:, k, :],
                                         start=(k == 0), stop=(k == EK - 1))
                        if k < EK - 1:
                            yield
                    vt_sb = vstage.tile([P, TCH], CD, tag="vt")
                    nc.vector.tensor_copy(vt_sb[:], acc[:])
                    yield
                    for i in range(TCH // P):
                        ps = pt.tile([P, P], CD, tag="tr")
                        nc.tensor.transpose(ps[:], vt_sb[:, i * P:(i + 1) * P],
                                            ident[:])
                        nc.vector.tensor_copy(vN[:, (t0 // P) + i, :], ps[:])
                        yield
                    for h in range(HQ):
                        acc = pa.tile([P, TCH], F32, tag="acc")
                        for k in range(EK):
                            nc.tensor.matmul(acc[:],
                                             wq_sb[:, k, h * P:(h + 1) * P],
                                             xt_t[:, k, :],
                                             start=(k == 0), stop=(k == EK - 1))
                            if k < EK - 1:
                                yield
                        nc.vector.tensor_copy(qT[:, h, t0:t0 + TCH], acc[:])
                        yield

                def drain(gen):
                    if gen is not None:
                        for _ in gen:
                            pass

                load_chunk(0)
                for ch in range(2):
                    drain(a_chunk_gen(ch))

                # second warmup: fires just before the first real AllToAll so
                # the collective engine doesn't re-idle in between
                warm2_in = dram.tile([N_CORES, 64], CD, tag="warm2_in")
                warm2_out = dram.tile([N_CORES, 64], CD, tag="warm2_out")
                nc.gpsimd.collective_compute(
                    "AllToAll", mybir.AluOpType.bypass,
                    ins=[warm2_in.opt()], outs=[warm2_out.opt()],
                    replica_groups=[list(range(N_CORES))])

                def a_tail():
                    for ch in range(2, NCH):
                        yield from a_chunk_gen(ch)

                def warm3(o_sb):
                    # data-dependent warmup: copying a sliver of B(0)'s
                    # mid-head output into the collective input delays the
                    # trigger until B(0) is actually half done on hardware,
                    # keeping the CC engines warm right before AllToAll(0)
                    w3i = dram.tile([N_CORES, 64], CD, tag="warm3_in")
                    w3o = dram.tile([N_CORES, 64], CD, tag="warm3_out")
                    nc.sync.dma_start(w3i[:], o_sb[0:N_CORES, 0:64])
                    nc.gpsimd.collective_compute(
                        "AllToAll", mybir.AluOpType.bypass,
                        ins=[w3i.opt()], outs=[w3o.opt()],
                        replica_groups=[list(range(N_CORES))])

                # pump_n=18 keeps every A-chunk's emission strictly ahead of
                # the B(0) chunks that read it (K/V/Q0 land in the first 100
                # yields of each 196-yield chunk); budget 1080 holds back the
                # final Q1-3 passes of chunk 7 (96 yields, not needed by B(0))
                # as early fill for B(1)'s first half
                tail = a_tail()
                emit_attention_head(0, pump=tail, pump_from=0, pump_n=18,
                                    pump_budget=1080, mid_cb=warm3, mid_at=5)
                # B(1) chunks 0-3: only needs qT[h=1]/kT/vN of chunks 0-3;
                # runs here so the A-tail leftover can fill its PE idle
                # (o_proj(0) can't — its AllToAll is still in flight)
                emit_attention_head(1, pump_early=tail, pump_from=4,
                                    chunks=range(0, SCB), fire_a2a=False)
                drain(tail)

            # ---- Region 2: heads 1-3 + o_proj pipeline (phase C) ----
            with tc.tile_pool(name="otp", bufs=2) as otp, \
                 tc.tile_pool(name="wop", bufs=9) as wop, \
                 tc.tile_pool(name="oaccp", bufs=1) as oaccp, \
                 tc.tile_pool(name="pc", bufs=3, space="PSUM") as pc:

                out_acc = oaccp.tile([P, MT, E], CD)

                def emit_oproj_loads(h):
                    wons = []
                    for q in range(NQ):
                        won_q = wop.tile([P, N_CORES, ECH], CD, tag="wo")
                        nc.sync.dma_start(won_q[:], wo[h, q])
                        wons.append(won_q)
                    return wons

                def emit_ot(h):
                    # gather this head's AllToAll result, split per m-tile so
                    # the first o_proj matmul only waits for a quarter of the
                    # data. The sync engine stalls on the collective's
                    # semaphore at enqueue time — ordering below ensures
                    # nothing urgent sits behind it in the sync queue.
                    ot_h = otp.tile([P, N_CORES, TSLICE], CD, tag="ot")
                    a2a_r = a2a_out[h][:].rearrange("(j p) t -> p j t", p=P)
                    for m in range(MT):
                        nc.sync.dma_start(ot_h[:, :, m * P:(m + 1) * P],
                                          a2a_r[:, :, m * P:(m + 1) * P])
                    return ot_h

                def emit_oproj_head(h, ot_h, wons, m_outer=False):
                    # q-chunks processed in pairs: within a (jj, m) step the
                    # two matmuls share the same stationary (the ot slice), so
                    # the compiler can skip the second LDWEIGHTS.
                    # m-outer (last head): each output row-tile finishes early
                    # so its final out-DMA overlaps the remaining matmuls
                    if m_outer:
                        order = [(q2, m) for m in range(MT)
                                 for q2 in range(0, NQ, 2)]
                    else:
                        order = [(q2, m) for q2 in range(0, NQ, 2)
                                 for m in range(MT)]

                    def gen():
                        for q2, m in order:
                            acc0 = pc.tile([P, ECH], F32, tag="c")
                            acc1 = pc.tile([P, ECH], F32, tag="c")
                            for jj in range(N_CORES):
                                ot_s = ot_h[:, jj, m * P:(m + 1) * P]
                                nc.tensor.matmul(
                                    acc0[:], ot_s, wons[q2][:, jj, :],
                                    start=(jj == 0), stop=(jj == N_CORES - 1))
                                yield
                                nc.tensor.matmul(
                                    acc1[:], ot_s, wons[q2 + 1][:, jj, :],
                                    start=(jj == 0), stop=(jj == N_CORES - 1))
                                yield
                            for i, acc_c in enumerate((acc0, acc1)):
                                q = q2 + i
                                if h == 0:
                                    nc.vector.tensor_copy(
                                        out_acc[:, m, q * ECH:(q + 1) * ECH],
                                        acc_c[:])
                                else:
                                    nc.vector.tensor_add(
                                        out_acc[:, m, q * ECH:(q + 1) * ECH],
                                        out_acc[:, m, q * ECH:(q + 1) * ECH],
                                        acc_c[:])
                                yield
                            # out-row halves fire as soon as their q-chunks
                            # are done, shrinking the post-compute DMA tail
                            if m_outer and q2 == NQ // 2 - 2:
                                nc.sync.dma_start(
                                    out[m * P:(m + 1) * P, :E // 2],
                                    out_acc[:, m, :E // 2])
                            if m_outer and q2 == NQ - 2:
                                nc.sync.dma_start(
                                    out[m * P:(m + 1) * P, E // 2:],
                                    out_acc[:, m, E // 2:])

                    return gen()

                def drain2(gen):
                    for _ in gen:
                        pass

                # Each C(h-1) gen is pumped into B(h) sc>=4 up to a budget;
                # its leftover becomes the early fill for B(h+1) sc<4 (where
                # C(h)'s own AllToAll is still in flight), and C(2)'s bigger
                # leftover covers the final AllToAll's latency.
                ot_p = emit_ot(0)       # AllToAll(0) already done: no stall
                wons = emit_oproj_loads(0)
                gen = emit_oproj_head(0, ot_p, wons)
                wons = emit_oproj_loads(1)
                # B(1) second half (first half ran under the A-tail leftover)
                emit_attention_head(1, pump=gen, pump_from=SCB + 1, pump_n=10,
                                    pump_budget=220, chunks=range(SCB, NSC))
                early = gen
                for h in range(2, HQ):
                    ot_p = emit_ot(h - 1)
                    gen = emit_oproj_head(h - 1, ot_p, wons)
                    wons = emit_oproj_loads(h)
                    budget = 130 if h == HQ - 1 else 220
                    emit_attention_head(h, pump=gen, pump_early=early,
                                        pump_from=5, pump_n=10,
                                        pump_budget=budget)
                    early = gen
                drain2(early)  # C(2) leftover runs while AllToAll(3) flies
                ot_p = emit_ot(HQ - 1)
                drain2(emit_oproj_head(HQ - 1, ot_p, wons, m_outer=True))

    nc.compile()
    return nc, c


def _make_masks(cfg):
    c = _derive(cfg)
    SCH = c["SCH"]
    NOFF = SCH // P
    m = np.zeros((NOFF, P, SCH), np.float32)
    for o in range(NOFF):
        for p in range(P):
            lo = p + o * P
            if lo < SCH:
                m[o, p, lo:] = 1.0
    return np.ascontiguousarray(m.transpose(1, 0, 2)).astype(BF)


def make_in_maps(cfg, hidden_states, Wq, Wk, Wv, Wo):
    c = _derive(cfg)
    B, S, E, NH, HQ = c["B"], c["S"], c["E"], c["NH"], c["HQ"]
    T, EK, TCH, ECH, NQ = c["T"], c["EK"], c["TCH"], c["ECH"], c["NQ"]
    NCH = T // TCH
    # X^T tiled [p, chunk, k, t'] so each chunk load is per-partition linear
    h2 = np.asarray(hidden_states, np.float32).reshape(T, E).astype(BF)
    xt_c = np.ascontiguousarray(
        h2.reshape(NCH, TCH, EK, P).transpose(3, 0, 2, 1))
    Wq = np.asarray(Wq, np.float32).astype(BF)
    Wk = np.asarray(Wk, np.float32).astype(BF)
    Wv = np.asarray(Wv, np.float32).astype(BF)
    # Wo [j*HQ*P + h*P + p, q*ECH + e'] -> [h, q, p, j*ECH + e']
    wo_c = np.ascontiguousarray(
        np.asarray(Wo, np.float32).astype(BF)
        .reshape(N_CORES, HQ, P, NQ, ECH).transpose(1, 3, 2, 0, 4)
        .reshape(HQ, NQ, P, N_CORES * ECH))
    masks = _make_masks(cfg)
    ones_col = np.ones((P, 1), BF)
    ones_row = np.ones((1, P), BF)
    ident = np.eye(P, dtype=np.float32).astype(BF)
    nkv_per = max(c["NKV"] // N_CORES, 1)
    in_maps = []
    for cidx in range(N_CORES):
        wq_c = np.ascontiguousarray(
            Wq[:, cidx * HQ:(cidx + 1) * HQ, :]
            .reshape(EK, P, HQ * P).transpose(1, 0, 2))
        wk_c = np.ascontiguousarray(
            Wk[:, cidx * nkv_per, :].reshape(EK, P, P).transpose(1, 0, 2))
        wv_c = np.ascontiguousarray(
            Wv[:, cidx * nkv_per, :].reshape(EK, P, P).transpose(1, 0, 2))
        in_maps.append({
            "xt": xt_c,
            "wq": wq_c,
            "wk": wk_c,
            "wv": wv_c,
            "wo": wo_c,
            "masks": masks,
            "ones_c": ones_col,
            "ones_r": ones_row,
            "identity": ident,
        })
    return in_maps


_CACHE = {}


def _get_built(key, cfg):
    if key not in _CACHE:
        _CACHE[key] = build(cfg)
    return _CACHE[key]


def kernel(hidden_states, Wq, Wk, Wv, Wo):
    cfg = FULL_CFG
    nc, c = _get_built("full", cfg)
    in_maps = make_in_maps(cfg, hidden_states, Wq, Wk, Wv, Wo)
    res = run_bass_kernel_spmd(nc, in_maps, core_ids=list(range(N_CORES)))
    outs = [np.asarray(res.results[i]["out"]).astype(np.float32)
            for i in range(N_CORES)]
    full = np.concatenate(outs, axis=0)
    return full.reshape(c["B"], c["S"], c["E"])
